# revision 46
# baseline (speedup 1.0000x reference)
"""Two-layer GAT on Trainium2, sharded over 8 NeuronCores.

Strategy:
  - Nodes split into 8 contiguous shards (6250/core, padded to a 128-aligned
    6272-row "row space" chunk per core); edges sorted by dst and owned by
    the core whose shard contains the dst.
  - Dense layer-1 projection is node-sharded (each core projects only its
    own chunk in bf16) producing haugL[r] = [h(256) | el(4) | er(4) | pad]
    (row = 384 elems = 768 B, a multiple of the 256 B SWDGE-gather granule)
    plus a compact elerL[r] = [el | er | pad] 256 B row; haugL is AllGathered
    into the full table.
  - Edge phase: per block of 128 consecutive dst nodes, all of the block's
    src rows are fetched with chunked InstDMAGatherAnt ops (int16 indices
    limit one gather to 32768 table rows -> lo/hi table views; the HW SWDGE
    ring caps one gather at 1024 rows -> <=8 columns per instruction);
    er[dst] needs only the core-local elerL (dst is in-shard by
    construction), one small gather per block. w = exp(leaky_relu(el+er))
    on-chip; segment-sum via per-column selection-matrix matmuls (bf16)
    accumulating in PSUM; softmax normalizes once per node after
    accumulation (exp without max-subtraction is safe: |e| << 88).
  - The layer-2 projection is fused into the layer-1 block loop; its table
    [6272, 128]bf16 is AllGathered and the layer-2 edge phase mirrors
    layer 1 (er2 comes from the local h2loc).
  - All per-core variability lives in uploaded metadata (indices, segids),
    so one SPMD program serves all 8 cores. Inputs are packed into few
    buffers and kept minimal (sharded features, 16-partition indices): this
    execution path re-ships every input byte per run (~13 GB/s), so input
    size directly bounds wall time.
"""

import numpy as np
import ml_dtypes

import concourse.bass as bass
import concourse.bacc as bacc
import concourse.mybir as mybir
import concourse.tile as tile
from concourse.bass import IndirectOffsetOnAxis
from concourse.bass_utils import run_bass_kernel_spmd

F32 = mybir.dt.float32
BF16 = mybir.dt.bfloat16
I16 = mybir.dt.int16
I32 = mybir.dt.int32
AF = mybir.ActivationFunctionType
OP = mybir.AluOpType

P = 128
NCORES = 8

# problem constants (hardcoded per spec)
N = 50000
E = 800000
IN = 256
HID = 64
OUT = 64
H1 = 4
H2 = 1
SLOPE = 0.2

F1 = H1 * HID          # 256
F2 = H2 * OUT          # 64
C1 = F1 + 2 * H1       # 264 real cols of layer-1 table row
C2 = F2 + 2 * H2       # 66 real cols of layer-2 table row
A1 = 384               # padded bf16 row width, layer-1 table (768 B)
A2 = 128               # padded bf16 row width, layer-2 table (256 B)
NLO = 32768            # int16 gather-index range split point
SHARD = N // NCORES    # 6250
NB = (SHARD + P - 1) // P  # 49
NTP = NCORES * NB      # 392 dense blocks (padded)
NPAD = NTP * P         # 50176 padded node rows
DSH = NPAD // NCORES   # 6272 dense rows per core


def _ceil_div(a, b):
    return (a + b - 1) // b


def _wrap_idx16(vals, ncols):
    """[n] values -> int16 [128, ncols] wrapped in 16 partitions (position i
    at [i%16, i//16]) and replicated x8 across the 128 partitions."""
    flat = np.zeros(16 * ncols, np.int16)
    flat[:len(vals)] = vals
    out = np.ascontiguousarray(flat.reshape(ncols, 16).T)
    return np.tile(out, (8, 1))


def preprocess_edges(src, dst):
    """Sort edges by dst, shard by dst range, split each (core, block)'s
    edges by src < NLO, pack into a uniform (core, block, column) grid.

    Returns per-core metadata arrays and (tlo, thi).
    """
    src = np.asarray(src).astype(np.int64)
    dst = np.asarray(dst).astype(np.int64)
    order = np.argsort(dst, kind="stable")
    ssrc = src[order]
    sdst = dst[order]

    # table rows live in "row space": node n -> n + (DSH-SHARD)*(n//SHARD),
    # i.e. each core's SHARD nodes padded to a 128-aligned DSH chunk
    srow = ssrc + (DSH - SHARD) * (ssrc // SHARD)

    core_of = sdst // SHARD
    block_of = (sdst % SHARD) // P
    is_lo = srow < NLO

    flat = (core_of * NB + block_of) * 2 + (~is_lo)
    counts = np.bincount(flat, minlength=NCORES * NB * 2)
    clo = counts[0::2].reshape(NCORES, NB)
    chi = counts[1::2].reshape(NCORES, NB)
    tlo = max(1, int(_ceil_div(clo.max(), P)))
    thi = max(1, int(_ceil_div(chi.max(), P)))
    t = tlo + thi

    # order edges by (core, block, hi/lo) so groups are contiguous
    order2 = np.lexsort((~is_lo, block_of, core_of))
    srow = srow[order2]
    sdst = sdst[order2]
    starts = np.zeros(NCORES * NB * 2 + 1, dtype=np.int64)
    np.cumsum(counts, out=starts[1:])

    idx16s, dloc16s, segids = [], [], []
    for c in range(NCORES):
        ix = np.zeros((16, NB * t * 8), np.int16)
        dc = np.zeros((16, NB * t * 8), np.int16)
        sg = np.full((P, NB * t), 512.0, np.float32)
        for b in range(NB):
            base = (c * NB + b) * 2
            for half, (toff, tcols) in enumerate(((0, tlo), (tlo, thi))):
                lo = starts[base + half]
                hi = starts[base + half + 1]
                cnt = hi - lo
                assert cnt <= tcols * P
                sval = srow[lo:hi] - (NLO if half else 0)
                dval = sdst[lo:hi]
                c0 = (b * t + toff) * 8
                c1 = (b * t + toff + tcols) * 8
                ix[:, c0:c1] = _wrap_idx16(sval, tcols * 8)[:16]
                dc[:, c0:c1] = _wrap_idx16(dval - c * SHARD, tcols * 8)[:16]
                s = np.arange(cnt)
                rows = s % P
                cols = b * t + toff + s // P
                sg[rows, cols] = (dval % SHARD) % P
        idx16s.append(ix)
        dloc16s.append(dc)
        segids.append(sg)
    return idx16s, dloc16s, segids, tlo, thi


def build_program(tlo, thi, skip_b1, skip_b2, nocoll=False, parts="full"):
    """Trace the SPMD Bass program. Returns nc.

    parts: "full" | "d1" (dense only) | "e1" (dense + layer-1 edge) |
           "e1g" (e1 but only src gathers, er zeroed) |
           "e1e" (e1 but src gathers skipped, er fetched) |
           "fullN<k>" (full but only k edge blocks per phase)"""
    nb_run = NB
    if parts.startswith("fullN"):
        nb_run = int(parts[5:])
        parts = "full"
    d1flags = ""
    if parts.startswith("d1x"):
        d1flags = parts[3:]
        parts = "d1"
    t = tlo + thi
    ki1 = IN // P   # 2
    ki2 = F1 // P   # 2
    nt1 = _ceil_div(N, P)  # 391

    nc = bacc.Bacc("TRN2", target_bir_lowering=False, debug=False,
                   num_devices=NCORES, num_swdge_queues=4)

    # features pre-blocked host-side and SHARDED: each core gets only its
    # own DSH dense rows (input staging over the axon tunnel is ~13 GB/s
    # aggregate and re-ships every input byte per execution, so input size
    # dominates wall time). Row nt*128+p holds x[base + nt*128 + n, k*128+p]
    # as one contiguous 64 KB region per block.
    xTb = nc.dram_tensor("xTb", [DSH, ki1 * P], BF16,
                         kind="ExternalInput").ap()
    S8 = NB * t * 8
    SB = NB * t
    # packed inputs (fewer device buffers -> less per-exec staging overhead):
    # [idx16 | dloc16 | metab([128, SB+P] bf16 wrapped into 8 i16 rows)]
    MW = SB + P
    meta16 = nc.dram_tensor("meta16", [16, 2 * S8 + 8 * MW], I16,
                            kind="ExternalInput").ap()
    idxs, dloc = meta16[:, 0:S8], meta16[:, S8:2 * S8]
    metab = (meta16[:, 2 * S8:2 * S8 + 8 * MW]
             .rearrange("q (r c) -> q r c", r=8).bitcast(BF16))
    wts = nc.dram_tensor("wts", [IN, C1 + C2], BF16, kind="ExternalInput").ap()
    w1a, w2a = wts[:, 0:C1], wts[:, C1:C1 + C2]
    bias = nc.dram_tensor("bias", [P, F1 + C2 + F2], F32,
                          kind="ExternalInput").ap()
    b1e, bc2, b2e = (bias[:, 0:F1], bias[:, F1:F1 + C2],
                     bias[:, F1 + C2:F1 + C2 + F2])
    outd = nc.dram_tensor("out", [SHARD, F2], BF16, kind="ExternalOutput").ap()

    haugL = nc.dram_tensor("haugL", [DSH, A1], BF16, kind="Internal").ap()
    # compact [el(4) | er(4) | pad] rows: full-row 256B gathers for er[dst]
    # (sub-row elem_step gathers return garbage on HW); local dst -> no
    # collective needed for layer-1 er
    elerL = nc.dram_tensor("elerL", [DSH, A2], BF16, kind="Internal").ap()
    haug = nc.dram_tensor("haug", [NPAD, A1], BF16, kind="Internal",
                          addr_space="Shared").ap()
    h2loc = nc.dram_tensor("h2loc", [DSH, A2], BF16, kind="Internal").ap()
    h2full = nc.dram_tensor("h2full", [NPAD, A2], BF16, kind="Internal",
                            addr_space="Shared").ap()

    with tile.TileContext(nc) as tc:
        with (
            tc.tile_pool(name="const", bufs=1) as cp,
            tc.tile_pool(name="xload", bufs=3) as xp,
            tc.tile_pool(name="stage", bufs=3) as sp,
            tc.tile_pool(name="gath", bufs=3) as gp,
            tc.tile_pool(name="g2p", bufs=3) as g2p,
            tc.tile_pool(name="small", bufs=3) as mp,
            tc.tile_pool(name="sel", bufs=3) as selp,
            tc.tile_pool(name="hwork", bufs=2) as hp,
            tc.tile_pool(name="psA", bufs=2, space="PSUM") as ppa,
            tc.tile_pool(name="psB", bufs=2, space="PSUM") as ppb,
            tc.tile_pool(name="psC", bufs=2, space="PSUM") as ppc,
        ):
            # ---- persistent constants / metadata ----
            w1sb = cp.tile([P, ki1, C1], BF16)
            nc.sync.dma_start(out=w1sb[:], in_=w1a.rearrange("(k p) n -> p k n", p=P))
            w2sb = cp.tile([P, ki2, C2], BF16)
            nc.sync.dma_start(out=w2sb[:], in_=w2a.rearrange("(k p) n -> p k n", p=P))
            b1sb = cp.tile([P, F1], F32)
            nc.sync.dma_start(out=b1sb[:], in_=b1e)
            bc2sb = cp.tile([P, C2], F32)
            nc.sync.dma_start(out=bc2sb[:], in_=bc2)
            b2sb = cp.tile([P, F2], F32)
            nc.sync.dma_start(out=b2sb[:], in_=b2e)

            from concourse.masks import make_identity
            idn = cp.tile([P, P], F32)
            make_identity(nc, idn[:])
            ixsb = cp.tile([P, NB * t * 8], I16)
            dcsb = cp.tile([P, NB * t * 8], I16)
            for r in range(8):
                nc.sync.dma_start(out=ixsb[16 * r:16 * (r + 1), :], in_=idxs)
                nc.sync.dma_start(out=dcsb[16 * r:16 * (r + 1), :], in_=dloc)
            mbt = cp.tile([P, SB + P], BF16)
            nc.sync.dma_start(out=mbt[:], in_=metab)
            sgsb = mbt[:, 0:SB]
            iosb = mbt[:, SB:SB + P]

            qctr = [0]

            def gather_cols(out_tile, in_ap, idx_tile, blk, toff, ncols,
                            elem, elem_step=None):
                """Chunked dma_gather of `ncols` 128-row columns starting at
                column `toff` of block `blk` (ring cap: <=8 cols/instr)."""
                done = 0
                while done < ncols:
                    cc = min(8, ncols - done)
                    col = blk * t + toff + done
                    nc.gpsimd.dma_gather(
                        out_ap=out_tile[:, toff + done:toff + done + cc, :],
                        in_ap=in_ap,
                        idxs_ap=idx_tile[:, col * 8:(col + cc) * 8],
                        num_idxs=cc * P, num_idxs_reg=cc * P,
                        elem_size=elem, elem_step=elem_step,
                        queue_num=qctr[0] % 4)
                    qctr[0] += 1
                    done += cc

            # ---- phase D1: sharded dense layer 1 -> haugL [DSH, A1] bf16,
            # then AllGather into the full tables ----
            for nt in range(NB):
                xt = xp.tile([P, ki1, P], BF16, tag="xt")
                if "l" not in d1flags:
                    nc.sync.dma_start(
                        out=xt[:],
                        in_=xTb[nt * P:(nt + 1) * P, :]
                            .rearrange("p (k n) -> p k n", k=ki1),
                    )
                else:
                    nc.vector.memset(xt[:], 0.0)
                st = sp.tile([P, C1], BF16, tag="st")
                if "m" not in d1flags:
                    ph = ppa.tile([P, C1], F32, space="PSUM", tag="acc")
                    for k in range(ki1):
                        nc.tensor.matmul(out=ph[:, :], lhsT=xt[:, k, :],
                                         rhs=w1sb[:, k, :],
                                         start=(k == 0), stop=(k == ki1 - 1))
                    nc.scalar.copy(out=st[:, :], in_=ph[:, :])
                if "h" not in d1flags:
                    nc.sync.dma_start(out=haugL[nt * P:(nt + 1) * P, 0:C1],
                                      in_=st[:, :])
                if "e" not in d1flags:
                    nc.sync.dma_start(out=elerL[nt * P:(nt + 1) * P, 0:2 * H1],
                                      in_=st[:, F1:F1 + 2 * H1])
            if parts != "d1":
                nc.gpsimd.collective_compute(
                    "AllGather", OP.bypass,
                    replica_groups=[list(range(NCORES))],
                    ins=[haugL], outs=[haug])

            if parts == "d1":
                zf = sp.tile([P, F2], BF16, tag="zf")
                nc.vector.memset(zf[:], 0.0)
                for b in range(NB):
                    m = min(P, SHARD - b * P)
                    nc.sync.dma_start(out=outd[b * P:b * P + m, :],
                                      in_=zf[:m, :])

            # ---- phase E1 (+ fused dense layer 2) per 128-dst-node block ----
            if nb_run < NB:
                zf = sp.tile([P, F2], BF16, tag="zf")
                nc.vector.memset(zf[:], 0.0)
                for b in range(NB):
                    m = min(P, SHARD - b * P)
                    nc.sync.dma_start(out=outd[b * P:b * P + m, :],
                                      in_=zf[:m, :])
            for b in range(nb_run if parts != "d1" else 0):
                m = min(P, SHARD - b * P)

                g = gp.tile([P, t, A1], BF16, tag="g1")
                if parts != "e1e":
                    gather_cols(g, haug[0:NLO, :], ixsb, b, 0, tlo, A1)
                    gather_cols(g, haug[NLO:NPAD, :], ixsb, b, tlo, thi, A1)
                else:
                    nc.vector.memset(g[:], 0.0)
                # er[dst] via one local full-row gather (dst is in-shard)
                gl = gp.tile([P, t, A2], BF16, tag="gl")
                if parts != "e1g":
                    gather_cols(gl, elerL, dcsb, b, 0, t, A2)
                else:
                    nc.vector.memset(gl[:], 0.0)

                # e = el[src] + er[dst]; leaky_relu; w = exp(e) -> el slot
                ere = mp.tile([P, t, H1], BF16, tag="ere")
                nc.vector.tensor_tensor(out=ere[:], in0=g[:, :, F1:F1 + H1],
                                        in1=gl[:, :, H1:2 * H1], op=OP.add)
                tmp = mp.tile([P, t, H1], BF16, tag="tmp")
                nc.vector.tensor_scalar_mul(out=tmp[:], in0=ere[:], scalar1=SLOPE)
                nc.vector.tensor_tensor(out=ere[:], in0=ere[:], in1=tmp[:],
                                        op=OP.max)
                nc.scalar.activation(out=g[:, :, F1:F1 + H1], in_=ere[:],
                                     func=AF.Exp)
                # h[src] *= w (per head)
                g4 = g[:, :, 0:F1].rearrange("p t (h d) -> p t h d", h=H1)
                wb = (g[:, :, F1:F1 + H1]
                      .rearrange("p t (h o) -> p t h o", o=1)
                      .to_broadcast([P, t, H1, HID]))
                nc.vector.tensor_tensor(out=g4, in0=g4, in1=wb, op=OP.mult)

                # selection matrices for all t columns in one op
                sel = selp.tile([P, t, P], BF16, tag="sel")
                in0 = (sgsb[:, b * t:(b + 1) * t]
                       .rearrange("p (f o) -> p f o", o=1).to_broadcast([P, t, P]))
                in1 = iosb.rearrange("p (o f) -> p o f", o=1).to_broadcast([P, t, P])
                nc.vector.tensor_tensor(out=sel[:], in0=in0, in1=in1,
                                        op=OP.is_equal)

                # segment-sum via matmuls accumulating in PSUM
                pb = ppa.tile([P, F1 + H1], F32, space="PSUM", tag="acc")
                for tt in range(t):
                    nc.tensor.matmul(out=pb[:], lhsT=sel[:, tt, :],
                                     rhs=g[:, tt, 0:F1 + H1],
                                     start=(tt == 0), stop=(tt == t - 1))

                # normalize: out = agg / denom (+eps keeps empty nodes at 0)
                den = mp.tile([P, H1], F32, tag="den")
                nc.vector.tensor_scalar_add(out=den[:m], in0=pb[:m, F1:F1 + H1],
                                            scalar1=1e-30)
                nc.vector.reciprocal(out=den[:m], in_=den[:m])
                h1t = hp.tile([P, F1], F32, tag="h1t")
                nc.vector.tensor_tensor(
                    out=h1t[:m].rearrange("p (h d) -> p h d", h=H1),
                    in0=pb[:m, 0:F1].rearrange("p (h d) -> p h d", h=H1),
                    in1=den[:m].rearrange("p (h o) -> p h o", o=1)
                        .to_broadcast([m, H1, HID]),
                    op=OP.mult)
                if not skip_b1:
                    nc.vector.tensor_tensor(out=h1t[:m], in0=h1t[:m],
                                            in1=b1sb[:m], op=OP.add)
                # elu+1 (the -1 is folded into bc2): max(h,0) + exp(min(h,0))
                te = hp.tile([P, F1], F32, tag="te")
                nc.vector.tensor_scalar_min(out=te[:m], in0=h1t[:m], scalar1=0.0)
                nc.scalar.activation(out=te[:m], in_=te[:m], func=AF.Exp)
                nc.vector.tensor_scalar_max(out=h1t[:m], in0=h1t[:m], scalar1=0.0)
                nc.vector.tensor_tensor(out=h1t[:m], in0=h1t[:m], in1=te[:m],
                                        op=OP.add)

                # transpose h1 and project: h2aug = h1 @ w2aug + bc2
                h1T = hp.tile([P, ki2, P], BF16, tag="h1T")
                for k in range(ki2):
                    pt = ppb.tile([P, P], F32, space="PSUM", tag="pt")
                    nc.tensor.transpose(out=pt[:, :m],
                                        in_=h1t[:m, k * P:(k + 1) * P],
                                        identity=idn[:m, :m])
                    nc.scalar.copy(out=h1T[:, k, :m], in_=pt[:, :m])
                p2 = ppc.tile([P, C2], F32, space="PSUM", tag="p2")
                for k in range(ki2):
                    nc.tensor.matmul(out=p2[:m, :], lhsT=h1T[:, k, :m],
                                     rhs=w2sb[:, k, :],
                                     start=(k == 0), stop=(k == ki2 - 1))
                st2 = sp.tile([P, C2], BF16, tag="st2")
                nc.vector.tensor_tensor(out=st2[:m, :], in0=p2[:m, :],
                                        in1=bc2sb[:m, :], op=OP.add)
                nc.sync.dma_start(out=h2loc[b * P:b * P + m, 0:C2], in_=st2[:m, :])
                if parts in ("e1", "e1g", "e1e"):
                    of = sp.tile([P, F2], BF16, tag="of")
                    nc.scalar.copy(out=of[:m, :], in_=st2[:m, 0:F2])
                    nc.sync.dma_start(out=outd[b * P:b * P + m, :],
                                      in_=of[:m, :])

            # ---- phase C2: all-gather layer-2 table ----
            run_e2 = parts == "full"
            if run_e2:
                if nocoll:
                    nc.sync.dma_start(out=h2full[0:SHARD, :], in_=h2loc)
                else:
                    nc.gpsimd.collective_compute(
                        "AllGather", OP.bypass,
                        replica_groups=[list(range(NCORES))],
                        ins=[h2loc], outs=[h2full])

            # ---- phase E2: layer-2 edge phase -> final output ----
            for b in range(nb_run if run_e2 else 0):
                m = min(P, SHARD - b * P)

                g2 = g2p.tile([P, t, A2], BF16, tag="g2")
                gather_cols(g2, h2full[0:NLO, :], ixsb, b, 0, tlo, A2)
                gather_cols(g2, h2full[NLO:NPAD, :], ixsb, b, tlo, thi, A2)
                gl2 = g2p.tile([P, t, A2], BF16, tag="gl2")
                gather_cols(gl2, h2loc, dcsb, b, 0, t, A2)
                er2 = mp.tile([P, t, H2], BF16, tag="er2")
                nc.vector.tensor_tensor(out=er2[:], in0=g2[:, :, F2:F2 + H2],
                                        in1=gl2[:, :, F2 + H2:F2 + 2 * H2],
                                        op=OP.add)
                tmp2 = mp.tile([P, t, H2], BF16, tag="tmp2")
                nc.vector.tensor_scalar_mul(out=tmp2[:], in0=er2[:], scalar1=SLOPE)
                nc.vector.tensor_tensor(out=er2[:], in0=er2[:], in1=tmp2[:],
                                        op=OP.max)
                nc.scalar.activation(out=er2[:], in_=er2[:], func=AF.Exp)
                # h2 *= w; denominator column <- w
                g2h = g2[:, :, 0:F2]
                w2b = er2[:].to_broadcast([P, t, F2])  # H2 == 1
                nc.vector.tensor_tensor(out=g2h, in0=g2h, in1=w2b, op=OP.mult)
                nc.scalar.copy(out=g2[:, :, F2:F2 + H2], in_=er2[:])

                sel = selp.tile([P, t, P], BF16, tag="sel")
                in0 = (sgsb[:, b * t:(b + 1) * t]
                       .rearrange("p (f o) -> p f o", o=1).to_broadcast([P, t, P]))
                in1 = iosb.rearrange("p (o f) -> p o f", o=1).to_broadcast([P, t, P])
                nc.vector.tensor_tensor(out=sel[:], in0=in0, in1=in1,
                                        op=OP.is_equal)

                pb2 = ppa.tile([P, F2 + H2], F32, space="PSUM", tag="acc")
                for tt in range(t):
                    nc.tensor.matmul(out=pb2[:], lhsT=sel[:, tt, :],
                                     rhs=g2[:, tt, 0:F2 + H2],
                                     start=(tt == 0), stop=(tt == t - 1))

                den2 = mp.tile([P, H2], F32, tag="den2")
                nc.vector.tensor_scalar_add(out=den2[:m], in0=pb2[:m, F2:F2 + H2],
                                            scalar1=1e-30)
                nc.vector.reciprocal(out=den2[:m], in_=den2[:m])
                of = sp.tile([P, F2], BF16, tag="of")
                nc.vector.tensor_tensor(
                    out=of[:m].rearrange("p (h d) -> p h d", h=H2),
                    in0=pb2[:m, 0:F2].rearrange("p (h d) -> p h d", h=H2),
                    in1=den2[:m].rearrange("p (h o) -> p h o", o=1)
                        .to_broadcast([m, H2, F2]),
                    op=OP.mult)
                if not skip_b2:
                    nc.vector.tensor_tensor(out=of[:m], in0=of[:m], in1=b2sb[:m],
                                            op=OP.add)
                nc.sync.dma_start(out=outd[b * P:b * P + m, :], in_=of[:m, :])

    nc.compile()
    return nc


def make_inputs(features, src, dst, W1, al1, ar1, b1, W2, al2, ar2, b2):
    """Host-side preprocessing: per-core input dicts.
    Returns (in_maps, tlo, thi, skip_b1, skip_b2)."""
    features = np.asarray(features, dtype=np.float32)
    W1 = np.asarray(W1, dtype=np.float32)
    W2 = np.asarray(W2, dtype=np.float32)
    al1 = np.asarray(al1, dtype=np.float32).reshape(H1, HID)
    ar1 = np.asarray(ar1, dtype=np.float32).reshape(H1, HID)
    al2 = np.asarray(al2, dtype=np.float32).reshape(H2, OUT)
    ar2 = np.asarray(ar2, dtype=np.float32).reshape(H2, OUT)
    b1 = np.asarray(b1, dtype=np.float32).reshape(-1)
    b2 = np.asarray(b2, dtype=np.float32).reshape(-1)

    def blockdiag(a, heads, d):
        m = np.zeros((heads * d, heads), dtype=np.float32)
        for h in range(heads):
            m[h * d:(h + 1) * d, h] = a[h]
        return m

    w1aug = np.concatenate(
        [W1, W1 @ blockdiag(al1, H1, HID), W1 @ blockdiag(ar1, H1, HID)],
        axis=1)  # [IN, C1]
    w2aug = np.concatenate(
        [W2, W2 @ blockdiag(al2, H2, OUT), W2 @ blockdiag(ar2, H2, OUT)],
        axis=1)  # [F1, C2]

    # elu's -1 shift folded through w2aug: (X-1)@W = X@W - colsum(W)
    bc2 = np.tile(-w2aug.sum(axis=0, keepdims=True), (P, 1)).astype(np.float32)
    b1e = np.tile(b1[None, :], (P, 1)).astype(np.float32)
    b2e = np.tile(b2[None, :], (P, 1)).astype(np.float32)
    iota = np.tile(np.arange(P, dtype=np.float32)[None, :], (P, 1))

    skip_b1 = not np.any(b1)
    skip_b2 = not np.any(b2)

    # pre-block features in row space: core c's chunk holds its SHARD nodes
    # padded to DSH rows; xTb[c*DSH + nt*128 + p, k*128 + n] =
    #   x[c*SHARD + nt*128 + n, k*128 + p]
    ki1 = IN // P
    fpad = np.zeros((NPAD, IN), np.float32)
    for c in range(NCORES):
        fpad[c * DSH:c * DSH + SHARD] = features[c * SHARD:(c + 1) * SHARD]
    # [nt, n, k, p] -> [nt, p, k, n]
    a = fpad.reshape(NTP, P, ki1, P).transpose(0, 3, 2, 1)
    xTb = np.ascontiguousarray(
        a.reshape(NPAD, ki1 * P)).astype(ml_dtypes.bfloat16)
    w1ab = np.ascontiguousarray(w1aug).astype(ml_dtypes.bfloat16)
    w2ab = np.ascontiguousarray(w2aug).astype(ml_dtypes.bfloat16)
    iotab = iota.astype(ml_dtypes.bfloat16)

    idx16s, dloc16s, segids, tlo, thi = preprocess_edges(src, dst)

    wts = np.concatenate([w1ab, w2ab], axis=1)
    bias = np.concatenate([b1e, bc2, b2e], axis=1).astype(np.float32)
    in_maps = []
    for c in range(NCORES):
        metab = np.ascontiguousarray(np.concatenate(
            [segids[c].astype(ml_dtypes.bfloat16), iotab], axis=1))
        mw = metab.shape[1]
        metab_i16 = metab.reshape(16, 8, mw).view(np.int16).reshape(16, 8 * mw)
        meta16 = np.concatenate([idx16s[c], dloc16s[c], metab_i16], axis=1)
        in_maps.append({
            "xTb": np.ascontiguousarray(xTb[c * DSH:(c + 1) * DSH]),
            "meta16": np.ascontiguousarray(meta16),
            "wts": np.ascontiguousarray(wts),
            "bias": np.ascontiguousarray(bias),
        })
    return in_maps, tlo, thi, skip_b1, skip_b2


def _run(features, src, dst, W1, al1, ar1, b1, W2, al2, ar2, b2, **spmd_kwargs):
    in_maps, tlo, thi, skip_b1, skip_b2 = make_inputs(
        features, src, dst, W1, al1, ar1, b1, W2, al2, ar2, b2)
    nc = build_program(tlo, thi, skip_b1, skip_b2)
    res = run_bass_kernel_spmd(nc, in_maps, core_ids=list(range(NCORES)),
                               **spmd_kwargs)
    out = np.concatenate([res.results[c]["out"] for c in range(NCORES)], axis=0)
    return np.asarray(out).astype(np.float32), res


def kernel(features, src, dst, W1, al1, ar1, b1, W2, al2, ar2, b2):
    out, _ = _run(features, src, dst, W1, al1, ar1, b1, W2, al2, ar2, b2)
    return out


def run_timed(features, src, dst, W1, al1, ar1, b1, W2, al2, ar2, b2,
              iters=5):
    """Run like kernel(), but keep inputs device-resident and time repeated
    executions of the compiled NEFF. Returns (out, best_wall_ns)."""
    import time as _time

    import jax
    from jax.sharding import Mesh, PartitionSpec
    from jax.experimental.shard_map import shard_map
    from concourse.bass2jax import (_bass_exec_p, install_neuronx_cc_hook,
                                    partition_id_tensor)

    in_maps, tlo, thi, skip_b1, skip_b2 = make_inputs(
        features, src, dst, W1, al1, ar1, b1, W2, al2, ar2, b2)
    nc = build_program(tlo, thi, skip_b1, skip_b2)

    install_neuronx_cc_hook()
    part_name = (nc.partition_id_tensor.name if nc.partition_id_tensor
                 else None)
    in_names, out_names, out_avals, zero_outs = [], [], [], []
    for alloc in nc.m.functions[0].allocations:
        if not isinstance(alloc, mybir.MemoryLocationSet):
            continue
        name = alloc.memorylocations[0].name
        if alloc.kind == "ExternalInput":
            if name != part_name:
                in_names.append(name)
        elif alloc.kind == "ExternalOutput":
            out_names.append(name)
            shp = tuple(alloc.tensor_shape)
            dt = mybir.dt.np(alloc.dtype)
            out_avals.append(jax.core.ShapedArray(shp, dt))
            zero_outs.append(np.zeros(shp, dt))
    n_params = len(in_names)
    all_names = in_names + out_names
    if part_name is not None:
        all_names = all_names + [part_name]

    def _body(*args):
        operands = list(args)
        if part_name is not None:
            operands.append(partition_id_tensor())
        return tuple(_bass_exec_p.bind(
            *operands, out_avals=tuple(out_avals), in_names=tuple(all_names),
            out_names=tuple(out_names), lowering_input_output_aliases=(),
            sim_require_finite=True, sim_require_nnan=True, nc=nc))

    devices = jax.devices()[:NCORES]
    mesh = Mesh(np.asarray(devices), ("core",))
    specs = (PartitionSpec("core"),) * (n_params + len(out_names))
    out_specs = (PartitionSpec("core"),) * len(out_names)
    fn = jax.jit(shard_map(_body, mesh=mesh, in_specs=specs,
                           out_specs=out_specs, check_rep=False),
                 keep_unused=True)

    concat_in = [np.concatenate([in_maps[c][nm] for c in range(NCORES)], axis=0)
                 for nm in in_names]
    concat_zero = [np.concatenate([z] * NCORES, axis=0) for z in zero_outs]
    args = [jax.device_put(a) for a in concat_in + concat_zero]
    outs = fn(*args)  # compile + warm up
    jax.block_until_ready(outs)

    def timed_chain(k):
        best = None
        for _ in range(iters):
            t0 = _time.perf_counter_ns()
            for _ in range(k):
                outs = fn(*args)
            jax.block_until_ready(outs)
            dt = _time.perf_counter_ns() - t0
            best = dt if best is None else min(best, dt)
        return best

    t1 = timed_chain(1)
    t11 = timed_chain(11)
    slope = max((t11 - t1) // 10, 1)
    print(f"[timing] 1-call wall: {t1/1e6:.2f} ms; 11-call wall: "
          f"{t11/1e6:.2f} ms; marginal per-exec: {slope/1e6:.3f} ms",
          flush=True)
    outs = fn(*args)
    jax.block_until_ready(outs)
    out_full = np.asarray(outs[out_names.index("out")])
    return out_full.astype(np.float32), slope


# revision 47
# speedup vs baseline: 1.1270x; 1.1270x over previous
"""Two-layer GAT on Trainium2, sharded over 8 NeuronCores.

Strategy:
  - Nodes split into 8 contiguous shards (6250/core, padded to a 128-aligned
    6272-row "row space" chunk per core); edges sorted by dst and owned by
    the core whose shard contains the dst.
  - Dense layer-1 projection is node-sharded (each core projects only its
    own chunk in bf16) producing haugL[r] = [h(256) | el(4) | er(4) | pad]
    (row = 384 elems = 768 B, a multiple of the 256 B SWDGE-gather granule)
    plus a compact elerL[r] = [el | er | pad] 256 B row; haugL is AllGathered
    into the full table.
  - Edge phase: per block of 128 consecutive dst nodes, all of the block's
    src rows are fetched with chunked InstDMAGatherAnt ops (int16 indices
    limit one gather to 32768 table rows -> lo/hi table views; the HW SWDGE
    ring caps one gather at 1024 rows -> <=8 columns per instruction);
    er[dst] needs only the core-local elerL (dst is in-shard by
    construction), one small gather per block. w = exp(leaky_relu(el+er))
    on-chip; segment-sum via per-column selection-matrix matmuls (bf16)
    accumulating in PSUM; softmax normalizes once per node after
    accumulation (exp without max-subtraction is safe: |e| << 88).
  - The layer-2 projection is fused into the layer-1 block loop; its table
    [6272, 128]bf16 is AllGathered and the layer-2 edge phase mirrors
    layer 1 (er2 comes from the local h2loc).
  - All per-core variability lives in uploaded metadata (indices, segids),
    so one SPMD program serves all 8 cores. Inputs are packed into few
    buffers and kept minimal (sharded features, 16-partition indices): this
    execution path re-ships every input byte per run (~13 GB/s), so input
    size directly bounds wall time.
"""

import numpy as np
import ml_dtypes

import concourse.bass as bass
import concourse.bacc as bacc
import concourse.mybir as mybir
import concourse.tile as tile
from concourse.bass import IndirectOffsetOnAxis
from concourse.bass_utils import run_bass_kernel_spmd

F32 = mybir.dt.float32
BF16 = mybir.dt.bfloat16
I16 = mybir.dt.int16
I32 = mybir.dt.int32
AF = mybir.ActivationFunctionType
OP = mybir.AluOpType

P = 128
NCORES = 8

# problem constants (hardcoded per spec)
N = 50000
E = 800000
IN = 256
HID = 64
OUT = 64
H1 = 4
H2 = 1
SLOPE = 0.2

F1 = H1 * HID          # 256
F2 = H2 * OUT          # 64
C1 = F1 + 2 * H1       # 264 real cols of layer-1 table row
C2 = F2 + 2 * H2       # 66 real cols of layer-2 table row
A1 = 384               # padded bf16 row width, layer-1 table (768 B)
A2 = 128               # padded bf16 row width, layer-2 table (256 B)
NLO = 32768            # int16 gather-index range split point
SHARD = N // NCORES    # 6250
NB = (SHARD + P - 1) // P  # 49
NTP = NCORES * NB      # 392 dense blocks (padded)
NPAD = NTP * P         # 50176 padded node rows
DSH = NPAD // NCORES   # 6272 dense rows per core


def _ceil_div(a, b):
    return (a + b - 1) // b


def _wrap_idx16(vals, ncols):
    """[n] values -> int16 [128, ncols] wrapped in 16 partitions (position i
    at [i%16, i//16]) and replicated x8 across the 128 partitions."""
    flat = np.zeros(16 * ncols, np.int16)
    flat[:len(vals)] = vals
    out = np.ascontiguousarray(flat.reshape(ncols, 16).T)
    return np.tile(out, (8, 1))


def preprocess_edges(src, dst):
    """Sort edges by dst, shard by dst range, split each (core, block)'s
    edges by src < NLO, pack into a uniform (core, block, column) grid.

    Returns per-core metadata arrays and (tlo, thi).
    """
    src = np.asarray(src).astype(np.int64)
    dst = np.asarray(dst).astype(np.int64)
    order = np.argsort(dst, kind="stable")
    ssrc = src[order]
    sdst = dst[order]

    # table rows live in "row space": node n -> n + (DSH-SHARD)*(n//SHARD),
    # i.e. each core's SHARD nodes padded to a 128-aligned DSH chunk
    srow = ssrc + (DSH - SHARD) * (ssrc // SHARD)

    core_of = sdst // SHARD
    block_of = (sdst % SHARD) // P
    is_lo = srow < NLO

    flat = (core_of * NB + block_of) * 2 + (~is_lo)
    counts = np.bincount(flat, minlength=NCORES * NB * 2)
    clo = counts[0::2].reshape(NCORES, NB)
    chi = counts[1::2].reshape(NCORES, NB)
    tlo = max(1, int(_ceil_div(clo.max(), P)))
    thi = max(1, int(_ceil_div(chi.max(), P)))
    t = tlo + thi

    # order edges by (core, block, hi/lo) so groups are contiguous
    order2 = np.lexsort((~is_lo, block_of, core_of))
    srow = srow[order2]
    sdst = sdst[order2]
    starts = np.zeros(NCORES * NB * 2 + 1, dtype=np.int64)
    np.cumsum(counts, out=starts[1:])

    idx16s, dloc16s, segids = [], [], []
    for c in range(NCORES):
        ix = np.zeros((16, NB * t * 8), np.int16)
        dc = np.zeros((16, NB * t * 8), np.int16)
        sg = np.full((P, NB * t), 512.0, np.float32)
        for b in range(NB):
            base = (c * NB + b) * 2
            for half, (toff, tcols) in enumerate(((0, tlo), (tlo, thi))):
                lo = starts[base + half]
                hi = starts[base + half + 1]
                cnt = hi - lo
                assert cnt <= tcols * P
                sval = srow[lo:hi] - (NLO if half else 0)
                dval = sdst[lo:hi]
                c0 = (b * t + toff) * 8
                c1 = (b * t + toff + tcols) * 8
                ix[:, c0:c1] = _wrap_idx16(sval, tcols * 8)[:16]
                dc[:, c0:c1] = _wrap_idx16(dval - c * SHARD, tcols * 8)[:16]
                s = np.arange(cnt)
                rows = s % P
                cols = b * t + toff + s // P
                sg[rows, cols] = (dval % SHARD) % P
        idx16s.append(ix)
        dloc16s.append(dc)
        segids.append(sg)
    return idx16s, dloc16s, segids, tlo, thi


def build_program(tlo, thi, skip_b1, skip_b2, nocoll=False, parts="full"):
    """Trace the SPMD Bass program. Returns nc.

    parts: "full" | "d1" (dense only) | "e1" (dense + layer-1 edge) |
           "e1g" (e1 but only src gathers, er zeroed) |
           "e1e" (e1 but src gathers skipped, er fetched) |
           "fullN<k>" (full but only k edge blocks per phase)"""
    nb_run = NB
    if parts.startswith("fullN"):
        nb_run = int(parts[5:])
        parts = "full"
    d1flags = ""
    if parts.startswith("d1x"):
        d1flags = parts[3:]
        parts = "d1"
    t = tlo + thi
    ki1 = IN // P   # 2
    ki2 = F1 // P   # 2
    nt1 = _ceil_div(N, P)  # 391

    nc = bacc.Bacc("TRN2", target_bir_lowering=False, debug=False,
                   num_devices=NCORES, num_swdge_queues=3)

    # features pre-blocked host-side and SHARDED: each core gets only its
    # own DSH dense rows (input staging over the axon tunnel is ~13 GB/s
    # aggregate and re-ships every input byte per execution, so input size
    # dominates wall time). Row nt*128+p holds x[base + nt*128 + n, k*128+p]
    # as one contiguous 64 KB region per block.
    xTb = nc.dram_tensor("xTb", [DSH, ki1 * P], BF16,
                         kind="ExternalInput").ap()
    S8 = NB * t * 8
    SB = NB * t
    # packed inputs (fewer device buffers -> less per-exec staging overhead):
    # [idx16 | dloc16 | metab([128, SB+P] bf16 wrapped into 8 i16 rows)]
    MW = SB + P
    meta16 = nc.dram_tensor("meta16", [16, 2 * S8 + 8 * MW], I16,
                            kind="ExternalInput").ap()
    idxs, dloc = meta16[:, 0:S8], meta16[:, S8:2 * S8]
    metab = (meta16[:, 2 * S8:2 * S8 + 8 * MW]
             .rearrange("q (r c) -> q r c", r=8).bitcast(BF16))
    wts = nc.dram_tensor("wts", [IN, C1 + C2], BF16, kind="ExternalInput").ap()
    w1a, w2a = wts[:, 0:C1], wts[:, C1:C1 + C2]
    bias = nc.dram_tensor("bias", [P, F1 + C2 + F2], F32,
                          kind="ExternalInput").ap()
    b1e, bc2, b2e = (bias[:, 0:F1], bias[:, F1:F1 + C2],
                     bias[:, F1 + C2:F1 + C2 + F2])
    outd = nc.dram_tensor("out", [SHARD, F2], BF16, kind="ExternalOutput").ap()

    haugL = nc.dram_tensor("haugL", [DSH, A1], BF16, kind="Internal").ap()
    # compact [el(4) | er(4) | pad] rows: full-row 256B gathers for er[dst]
    # (sub-row elem_step gathers return garbage on HW); local dst -> no
    # collective needed for layer-1 er
    elerL = nc.dram_tensor("elerL", [DSH, A2], BF16, kind="Internal").ap()
    haug = nc.dram_tensor("haug", [NPAD, A1], BF16, kind="Internal",
                          addr_space="Shared").ap()
    h2loc = nc.dram_tensor("h2loc", [DSH, A2], BF16, kind="Internal").ap()
    h2full = nc.dram_tensor("h2full", [NPAD, A2], BF16, kind="Internal",
                            addr_space="Shared").ap()

    with tile.TileContext(nc) as tc:
        with (
            tc.tile_pool(name="const", bufs=1) as cp,
            tc.tile_pool(name="xload", bufs=3) as xp,
            tc.tile_pool(name="stage", bufs=3) as sp,
            tc.tile_pool(name="gath", bufs=2) as gp,
            tc.tile_pool(name="g2p", bufs=2) as g2p,
            tc.tile_pool(name="small", bufs=3) as mp,
            tc.tile_pool(name="sel", bufs=2) as selp,
            tc.tile_pool(name="hwork", bufs=2) as hp,
            tc.tile_pool(name="psA", bufs=2, space="PSUM") as ppa,
            tc.tile_pool(name="psB", bufs=2, space="PSUM") as ppb,
            tc.tile_pool(name="psC", bufs=2, space="PSUM") as ppc,
        ):
            # ---- persistent constants / metadata ----
            w1sb = cp.tile([P, ki1, C1], BF16)
            nc.sync.dma_start(out=w1sb[:], in_=w1a.rearrange("(k p) n -> p k n", p=P))
            w2sb = cp.tile([P, ki2, C2], BF16)
            nc.sync.dma_start(out=w2sb[:], in_=w2a.rearrange("(k p) n -> p k n", p=P))
            b1sb = cp.tile([P, F1], F32)
            nc.sync.dma_start(out=b1sb[:], in_=b1e)
            bc2sb = cp.tile([P, C2], F32)
            nc.sync.dma_start(out=bc2sb[:], in_=bc2)
            b2sb = cp.tile([P, F2], F32)
            nc.sync.dma_start(out=b2sb[:], in_=b2e)

            from concourse.masks import make_identity
            idn = cp.tile([P, P], F32)
            make_identity(nc, idn[:])
            ixsb = cp.tile([P, NB * t * 8], I16)
            dcsb = cp.tile([P, NB * t * 8], I16)
            for r in range(8):
                nc.sync.dma_start(out=ixsb[16 * r:16 * (r + 1), :], in_=idxs)
                nc.sync.dma_start(out=dcsb[16 * r:16 * (r + 1), :], in_=dloc)
            mbt = cp.tile([P, SB + P], BF16)
            nc.sync.dma_start(out=mbt[:], in_=metab)
            sgsb = mbt[:, 0:SB]
            iosb = mbt[:, SB:SB + P]

            qctr = [0]

            def gather_cols(out_tile, in_ap, idx_tile, blk, toff, ncols,
                            elem, elem_step=None):
                """Chunked dma_gather of `ncols` 128-row columns starting at
                column `toff` of block `blk` (ring cap: <=8 cols/instr)."""
                done = 0
                while done < ncols:
                    cc = min(8, ncols - done)
                    col = blk * t + toff + done
                    nc.gpsimd.dma_gather(
                        out_ap=out_tile[:, toff + done:toff + done + cc, :],
                        in_ap=in_ap,
                        idxs_ap=idx_tile[:, col * 8:(col + cc) * 8],
                        num_idxs=cc * P, num_idxs_reg=cc * P,
                        elem_size=elem, elem_step=elem_step,
                        queue_num=qctr[0] % 3)
                    qctr[0] += 1
                    done += cc

            # ---- phase D1: sharded dense layer 1 -> haugL [DSH, A1] bf16,
            # then AllGather into the full tables ----
            for nt in range(NB):
                xt = xp.tile([P, ki1, P], BF16, tag="xt")
                if "l" not in d1flags:
                    nc.sync.dma_start(
                        out=xt[:],
                        in_=xTb[nt * P:(nt + 1) * P, :]
                            .rearrange("p (k n) -> p k n", k=ki1),
                    )
                else:
                    nc.vector.memset(xt[:], 0.0)
                st = sp.tile([P, C1], BF16, tag="st")
                if "m" not in d1flags:
                    ph = ppa.tile([P, C1], F32, space="PSUM", tag="acc")
                    for k in range(ki1):
                        nc.tensor.matmul(out=ph[:, :], lhsT=xt[:, k, :],
                                         rhs=w1sb[:, k, :],
                                         start=(k == 0), stop=(k == ki1 - 1))
                    nc.scalar.copy(out=st[:, :], in_=ph[:, :])
                if "h" not in d1flags:
                    nc.sync.dma_start(out=haugL[nt * P:(nt + 1) * P, 0:C1],
                                      in_=st[:, :])
                if "e" not in d1flags:
                    nc.sync.dma_start(out=elerL[nt * P:(nt + 1) * P, 0:2 * H1],
                                      in_=st[:, F1:F1 + 2 * H1])
            if parts != "d1":
                nc.gpsimd.collective_compute(
                    "AllGather", OP.bypass,
                    replica_groups=[list(range(NCORES))],
                    ins=[haugL], outs=[haug])

            if parts == "d1":
                zf = sp.tile([P, F2], BF16, tag="zf")
                nc.vector.memset(zf[:], 0.0)
                for b in range(NB):
                    m = min(P, SHARD - b * P)
                    nc.sync.dma_start(out=outd[b * P:b * P + m, :],
                                      in_=zf[:m, :])

            # ---- phase E1 (+ fused dense layer 2) per 128-dst-node block ----
            if nb_run < NB:
                zf = sp.tile([P, F2], BF16, tag="zf")
                nc.vector.memset(zf[:], 0.0)
                for b in range(NB):
                    m = min(P, SHARD - b * P)
                    nc.sync.dma_start(out=outd[b * P:b * P + m, :],
                                      in_=zf[:m, :])
            for b in range(nb_run if parts != "d1" else 0):
                m = min(P, SHARD - b * P)

                g = gp.tile([P, t, A1], BF16, tag="g1")
                if parts != "e1e":
                    gather_cols(g, haug[0:NLO, :], ixsb, b, 0, tlo, A1)
                    gather_cols(g, haug[NLO:NPAD, :], ixsb, b, tlo, thi, A1)
                else:
                    nc.vector.memset(g[:], 0.0)
                # er[dst] via one local full-row gather (dst is in-shard)
                gl = gp.tile([P, t, A2], BF16, tag="gl")
                if parts != "e1g":
                    gather_cols(gl, elerL, dcsb, b, 0, t, A2)
                else:
                    nc.vector.memset(gl[:], 0.0)

                # e = el[src] + er[dst]; leaky_relu; w = exp(e) -> el slot
                ere = mp.tile([P, t, H1], BF16, tag="ere")
                nc.vector.tensor_tensor(out=ere[:], in0=g[:, :, F1:F1 + H1],
                                        in1=gl[:, :, H1:2 * H1], op=OP.add)
                tmp = mp.tile([P, t, H1], BF16, tag="tmp")
                nc.vector.tensor_scalar_mul(out=tmp[:], in0=ere[:], scalar1=SLOPE)
                nc.vector.tensor_tensor(out=ere[:], in0=ere[:], in1=tmp[:],
                                        op=OP.max)
                nc.scalar.activation(out=g[:, :, F1:F1 + H1], in_=ere[:],
                                     func=AF.Exp)
                # h[src] *= w (per head)
                g4 = g[:, :, 0:F1].rearrange("p t (h d) -> p t h d", h=H1)
                wb = (g[:, :, F1:F1 + H1]
                      .rearrange("p t (h o) -> p t h o", o=1)
                      .to_broadcast([P, t, H1, HID]))
                nc.vector.tensor_tensor(out=g4, in0=g4, in1=wb, op=OP.mult)

                # selection matrices for all t columns in one op
                sel = selp.tile([P, t, P], BF16, tag="sel")
                in0 = (sgsb[:, b * t:(b + 1) * t]
                       .rearrange("p (f o) -> p f o", o=1).to_broadcast([P, t, P]))
                in1 = iosb.rearrange("p (o f) -> p o f", o=1).to_broadcast([P, t, P])
                nc.vector.tensor_tensor(out=sel[:], in0=in0, in1=in1,
                                        op=OP.is_equal)

                # segment-sum via matmuls accumulating in PSUM
                pb = ppa.tile([P, F1 + H1], F32, space="PSUM", tag="acc")
                for tt in range(t):
                    nc.tensor.matmul(out=pb[:], lhsT=sel[:, tt, :],
                                     rhs=g[:, tt, 0:F1 + H1],
                                     start=(tt == 0), stop=(tt == t - 1))

                # normalize: out = agg / denom (+eps keeps empty nodes at 0)
                den = mp.tile([P, H1], F32, tag="den")
                nc.vector.tensor_scalar_add(out=den[:m], in0=pb[:m, F1:F1 + H1],
                                            scalar1=1e-30)
                nc.vector.reciprocal(out=den[:m], in_=den[:m])
                h1t = hp.tile([P, F1], F32, tag="h1t")
                nc.vector.tensor_tensor(
                    out=h1t[:m].rearrange("p (h d) -> p h d", h=H1),
                    in0=pb[:m, 0:F1].rearrange("p (h d) -> p h d", h=H1),
                    in1=den[:m].rearrange("p (h o) -> p h o", o=1)
                        .to_broadcast([m, H1, HID]),
                    op=OP.mult)
                if not skip_b1:
                    nc.vector.tensor_tensor(out=h1t[:m], in0=h1t[:m],
                                            in1=b1sb[:m], op=OP.add)
                # elu+1 (the -1 is folded into bc2): max(h,0) + exp(min(h,0))
                te = hp.tile([P, F1], F32, tag="te")
                nc.vector.tensor_scalar_min(out=te[:m], in0=h1t[:m], scalar1=0.0)
                nc.scalar.activation(out=te[:m], in_=te[:m], func=AF.Exp)
                nc.vector.tensor_scalar_max(out=h1t[:m], in0=h1t[:m], scalar1=0.0)
                nc.vector.tensor_tensor(out=h1t[:m], in0=h1t[:m], in1=te[:m],
                                        op=OP.add)

                # transpose h1 and project: h2aug = h1 @ w2aug + bc2
                h1T = hp.tile([P, ki2, P], BF16, tag="h1T")
                for k in range(ki2):
                    pt = ppb.tile([P, P], F32, space="PSUM", tag="pt")
                    nc.tensor.transpose(out=pt[:, :m],
                                        in_=h1t[:m, k * P:(k + 1) * P],
                                        identity=idn[:m, :m])
                    nc.scalar.copy(out=h1T[:, k, :m], in_=pt[:, :m])
                p2 = ppc.tile([P, C2], F32, space="PSUM", tag="p2")
                for k in range(ki2):
                    nc.tensor.matmul(out=p2[:m, :], lhsT=h1T[:, k, :m],
                                     rhs=w2sb[:, k, :],
                                     start=(k == 0), stop=(k == ki2 - 1))
                st2 = sp.tile([P, C2], BF16, tag="st2")
                nc.vector.tensor_tensor(out=st2[:m, :], in0=p2[:m, :],
                                        in1=bc2sb[:m, :], op=OP.add)
                nc.sync.dma_start(out=h2loc[b * P:b * P + m, 0:C2], in_=st2[:m, :])
                if parts in ("e1", "e1g", "e1e"):
                    of = sp.tile([P, F2], BF16, tag="of")
                    nc.scalar.copy(out=of[:m, :], in_=st2[:m, 0:F2])
                    nc.sync.dma_start(out=outd[b * P:b * P + m, :],
                                      in_=of[:m, :])

            # ---- phase C2: all-gather layer-2 table ----
            run_e2 = parts == "full"
            if run_e2:
                if nocoll:
                    nc.sync.dma_start(out=h2full[0:SHARD, :], in_=h2loc)
                else:
                    nc.gpsimd.collective_compute(
                        "AllGather", OP.bypass,
                        replica_groups=[list(range(NCORES))],
                        ins=[h2loc], outs=[h2full])

            # ---- phase E2: layer-2 edge phase -> final output ----
            for b in range(nb_run if run_e2 else 0):
                m = min(P, SHARD - b * P)

                g2 = g2p.tile([P, t, A2], BF16, tag="g2")
                gather_cols(g2, h2full[0:NLO, :], ixsb, b, 0, tlo, A2)
                gather_cols(g2, h2full[NLO:NPAD, :], ixsb, b, tlo, thi, A2)
                gl2 = g2p.tile([P, t, A2], BF16, tag="gl2")
                gather_cols(gl2, h2loc, dcsb, b, 0, t, A2)
                er2 = mp.tile([P, t, H2], BF16, tag="er2")
                nc.vector.tensor_tensor(out=er2[:], in0=g2[:, :, F2:F2 + H2],
                                        in1=gl2[:, :, F2 + H2:F2 + 2 * H2],
                                        op=OP.add)
                tmp2 = mp.tile([P, t, H2], BF16, tag="tmp2")
                nc.vector.tensor_scalar_mul(out=tmp2[:], in0=er2[:], scalar1=SLOPE)
                nc.vector.tensor_tensor(out=er2[:], in0=er2[:], in1=tmp2[:],
                                        op=OP.max)
                nc.scalar.activation(out=er2[:], in_=er2[:], func=AF.Exp)
                # h2 *= w; denominator column <- w
                g2h = g2[:, :, 0:F2]
                w2b = er2[:].to_broadcast([P, t, F2])  # H2 == 1
                nc.vector.tensor_tensor(out=g2h, in0=g2h, in1=w2b, op=OP.mult)
                nc.scalar.copy(out=g2[:, :, F2:F2 + H2], in_=er2[:])

                sel = selp.tile([P, t, P], BF16, tag="sel")
                in0 = (sgsb[:, b * t:(b + 1) * t]
                       .rearrange("p (f o) -> p f o", o=1).to_broadcast([P, t, P]))
                in1 = iosb.rearrange("p (o f) -> p o f", o=1).to_broadcast([P, t, P])
                nc.vector.tensor_tensor(out=sel[:], in0=in0, in1=in1,
                                        op=OP.is_equal)

                pb2 = ppa.tile([P, F2 + H2], F32, space="PSUM", tag="acc")
                for tt in range(t):
                    nc.tensor.matmul(out=pb2[:], lhsT=sel[:, tt, :],
                                     rhs=g2[:, tt, 0:F2 + H2],
                                     start=(tt == 0), stop=(tt == t - 1))

                den2 = mp.tile([P, H2], F32, tag="den2")
                nc.vector.tensor_scalar_add(out=den2[:m], in0=pb2[:m, F2:F2 + H2],
                                            scalar1=1e-30)
                nc.vector.reciprocal(out=den2[:m], in_=den2[:m])
                of = sp.tile([P, F2], BF16, tag="of")
                nc.vector.tensor_tensor(
                    out=of[:m].rearrange("p (h d) -> p h d", h=H2),
                    in0=pb2[:m, 0:F2].rearrange("p (h d) -> p h d", h=H2),
                    in1=den2[:m].rearrange("p (h o) -> p h o", o=1)
                        .to_broadcast([m, H2, F2]),
                    op=OP.mult)
                if not skip_b2:
                    nc.vector.tensor_tensor(out=of[:m], in0=of[:m], in1=b2sb[:m],
                                            op=OP.add)
                nc.sync.dma_start(out=outd[b * P:b * P + m, :], in_=of[:m, :])

    nc.compile()
    return nc


def make_inputs(features, src, dst, W1, al1, ar1, b1, W2, al2, ar2, b2):
    """Host-side preprocessing: per-core input dicts.
    Returns (in_maps, tlo, thi, skip_b1, skip_b2)."""
    features = np.asarray(features, dtype=np.float32)
    W1 = np.asarray(W1, dtype=np.float32)
    W2 = np.asarray(W2, dtype=np.float32)
    al1 = np.asarray(al1, dtype=np.float32).reshape(H1, HID)
    ar1 = np.asarray(ar1, dtype=np.float32).reshape(H1, HID)
    al2 = np.asarray(al2, dtype=np.float32).reshape(H2, OUT)
    ar2 = np.asarray(ar2, dtype=np.float32).reshape(H2, OUT)
    b1 = np.asarray(b1, dtype=np.float32).reshape(-1)
    b2 = np.asarray(b2, dtype=np.float32).reshape(-1)

    def blockdiag(a, heads, d):
        m = np.zeros((heads * d, heads), dtype=np.float32)
        for h in range(heads):
            m[h * d:(h + 1) * d, h] = a[h]
        return m

    w1aug = np.concatenate(
        [W1, W1 @ blockdiag(al1, H1, HID), W1 @ blockdiag(ar1, H1, HID)],
        axis=1)  # [IN, C1]
    w2aug = np.concatenate(
        [W2, W2 @ blockdiag(al2, H2, OUT), W2 @ blockdiag(ar2, H2, OUT)],
        axis=1)  # [F1, C2]

    # elu's -1 shift folded through w2aug: (X-1)@W = X@W - colsum(W)
    bc2 = np.tile(-w2aug.sum(axis=0, keepdims=True), (P, 1)).astype(np.float32)
    b1e = np.tile(b1[None, :], (P, 1)).astype(np.float32)
    b2e = np.tile(b2[None, :], (P, 1)).astype(np.float32)
    iota = np.tile(np.arange(P, dtype=np.float32)[None, :], (P, 1))

    skip_b1 = not np.any(b1)
    skip_b2 = not np.any(b2)

    # pre-block features in row space: core c's chunk holds its SHARD nodes
    # padded to DSH rows; xTb[c*DSH + nt*128 + p, k*128 + n] =
    #   x[c*SHARD + nt*128 + n, k*128 + p]
    ki1 = IN // P
    fpad = np.zeros((NPAD, IN), np.float32)
    for c in range(NCORES):
        fpad[c * DSH:c * DSH + SHARD] = features[c * SHARD:(c + 1) * SHARD]
    # [nt, n, k, p] -> [nt, p, k, n]
    a = fpad.reshape(NTP, P, ki1, P).transpose(0, 3, 2, 1)
    xTb = np.ascontiguousarray(
        a.reshape(NPAD, ki1 * P)).astype(ml_dtypes.bfloat16)
    w1ab = np.ascontiguousarray(w1aug).astype(ml_dtypes.bfloat16)
    w2ab = np.ascontiguousarray(w2aug).astype(ml_dtypes.bfloat16)
    iotab = iota.astype(ml_dtypes.bfloat16)

    idx16s, dloc16s, segids, tlo, thi = preprocess_edges(src, dst)

    wts = np.concatenate([w1ab, w2ab], axis=1)
    bias = np.concatenate([b1e, bc2, b2e], axis=1).astype(np.float32)
    in_maps = []
    for c in range(NCORES):
        metab = np.ascontiguousarray(np.concatenate(
            [segids[c].astype(ml_dtypes.bfloat16), iotab], axis=1))
        mw = metab.shape[1]
        metab_i16 = metab.reshape(16, 8, mw).view(np.int16).reshape(16, 8 * mw)
        meta16 = np.concatenate([idx16s[c], dloc16s[c], metab_i16], axis=1)
        in_maps.append({
            "xTb": np.ascontiguousarray(xTb[c * DSH:(c + 1) * DSH]),
            "meta16": np.ascontiguousarray(meta16),
            "wts": np.ascontiguousarray(wts),
            "bias": np.ascontiguousarray(bias),
        })
    return in_maps, tlo, thi, skip_b1, skip_b2


def _run(features, src, dst, W1, al1, ar1, b1, W2, al2, ar2, b2, **spmd_kwargs):
    in_maps, tlo, thi, skip_b1, skip_b2 = make_inputs(
        features, src, dst, W1, al1, ar1, b1, W2, al2, ar2, b2)
    nc = build_program(tlo, thi, skip_b1, skip_b2)
    res = run_bass_kernel_spmd(nc, in_maps, core_ids=list(range(NCORES)),
                               **spmd_kwargs)
    out = np.concatenate([res.results[c]["out"] for c in range(NCORES)], axis=0)
    return np.asarray(out).astype(np.float32), res


def kernel(features, src, dst, W1, al1, ar1, b1, W2, al2, ar2, b2):
    out, _ = _run(features, src, dst, W1, al1, ar1, b1, W2, al2, ar2, b2)
    return out


def run_timed(features, src, dst, W1, al1, ar1, b1, W2, al2, ar2, b2,
              iters=5):
    """Run like kernel(), but keep inputs device-resident and time repeated
    executions of the compiled NEFF. Returns (out, best_wall_ns)."""
    import time as _time

    import jax
    from jax.sharding import Mesh, PartitionSpec
    from jax.experimental.shard_map import shard_map
    from concourse.bass2jax import (_bass_exec_p, install_neuronx_cc_hook,
                                    partition_id_tensor)

    in_maps, tlo, thi, skip_b1, skip_b2 = make_inputs(
        features, src, dst, W1, al1, ar1, b1, W2, al2, ar2, b2)
    nc = build_program(tlo, thi, skip_b1, skip_b2)

    install_neuronx_cc_hook()
    part_name = (nc.partition_id_tensor.name if nc.partition_id_tensor
                 else None)
    in_names, out_names, out_avals, zero_outs = [], [], [], []
    for alloc in nc.m.functions[0].allocations:
        if not isinstance(alloc, mybir.MemoryLocationSet):
            continue
        name = alloc.memorylocations[0].name
        if alloc.kind == "ExternalInput":
            if name != part_name:
                in_names.append(name)
        elif alloc.kind == "ExternalOutput":
            out_names.append(name)
            shp = tuple(alloc.tensor_shape)
            dt = mybir.dt.np(alloc.dtype)
            out_avals.append(jax.core.ShapedArray(shp, dt))
            zero_outs.append(np.zeros(shp, dt))
    n_params = len(in_names)
    all_names = in_names + out_names
    if part_name is not None:
        all_names = all_names + [part_name]

    def _body(*args):
        operands = list(args)
        if part_name is not None:
            operands.append(partition_id_tensor())
        return tuple(_bass_exec_p.bind(
            *operands, out_avals=tuple(out_avals), in_names=tuple(all_names),
            out_names=tuple(out_names), lowering_input_output_aliases=(),
            sim_require_finite=True, sim_require_nnan=True, nc=nc))

    devices = jax.devices()[:NCORES]
    mesh = Mesh(np.asarray(devices), ("core",))
    specs = (PartitionSpec("core"),) * (n_params + len(out_names))
    out_specs = (PartitionSpec("core"),) * len(out_names)
    fn = jax.jit(shard_map(_body, mesh=mesh, in_specs=specs,
                           out_specs=out_specs, check_rep=False),
                 keep_unused=True)

    concat_in = [np.concatenate([in_maps[c][nm] for c in range(NCORES)], axis=0)
                 for nm in in_names]
    concat_zero = [np.concatenate([z] * NCORES, axis=0) for z in zero_outs]
    args = [jax.device_put(a) for a in concat_in + concat_zero]
    outs = fn(*args)  # compile + warm up
    jax.block_until_ready(outs)

    def timed_chain(k):
        best = None
        for _ in range(iters):
            t0 = _time.perf_counter_ns()
            for _ in range(k):
                outs = fn(*args)
            jax.block_until_ready(outs)
            dt = _time.perf_counter_ns() - t0
            best = dt if best is None else min(best, dt)
        return best

    t1 = timed_chain(1)
    t11 = timed_chain(11)
    slope = max((t11 - t1) // 10, 1)
    print(f"[timing] 1-call wall: {t1/1e6:.2f} ms; 11-call wall: "
          f"{t11/1e6:.2f} ms; marginal per-exec: {slope/1e6:.3f} ms",
          flush=True)
    outs = fn(*args)
    jax.block_until_ready(outs)
    out_full = np.asarray(outs[out_names.index("out")])
    return out_full.astype(np.float32), slope
